# revision 42
# baseline (speedup 1.0000x reference)
"""MoTAttention Trainium2 kernel (self-contained).

B,L,D,H = 2,1024,768,12 ; d=64.  8 cores = (batch b in {0,1}) x (4 l-chunks of 256).
SPMD: one program; per-core data differs. Each core's hidden_states are ROLLED so
its own l-chunk sits at rows 0:256; rel's r-axis and the mask are rolled
identically (softmax/PV are invariant to a consistent permutation of r).

Phased structure (phases keep big-packet and small-packet DMA flows apart,
which the SDMA engines reward):
  P:  projections qT, rq -> LHS2; kT, V+ones, rk -> LHS3 (PSUM->SBUF copies
      split between the Vector and Scalar engines).
  S2: S2[h,l,r] l-pair matmuls (K=128 = 2l x 64d, M=32, 4 pairs/PSUM via
      col-group tile_position), rel streamed fp8e3 as 1MB partition-contiguous
      super-groups; dump bf16 [hs, l, r] (transposed reload needs 2B dtype).
  S3: same per r-pair, 2 dump-groups per PSUM bank; dump fp8e4 to
      s3dumpT == [r, hs, l] (phase-C reads are partition-contiguous 3KB rows).
  C:  per r-tile of 128: DMA-transpose reload of S2 (-> [r, (h,l)]) on the
      sync ring (idle in this phase), plain reload of S3; per head PSUM =
      S1 (q.kT head-sliced matmul) + 2I@s2r + 2I@s3r (tensor-engine adds;
      wkN/wqN are halved on host so S2/S3 fit fp8 range, the 2*I re-doubles);
      probs = exp(PSUM*SCALE + mask_r) straight off PSUM; PV with ones-column
      accumulates unnormalized attT + Z into SBUF.
  Tail: normalize via broadcast + partition-parallel reciprocal; output
      projection.
Host: layout prep only (casts/rolls/transposes) + exact bias fold
      (out += Wo@bv + bo; valid because softmax rows sum to 1).
"""

import math
import numpy as np
import ml_dtypes

BF16 = ml_dtypes.bfloat16
FP8 = ml_dtypes.float8_e3m4
B, L, D, H = 2, 1024, 768, 12
d = 64
LC = 256
NCORES = 8
ET = D // 128
SCALE = 1.0 / math.sqrt(3 * d)

_PROG_CACHE = {}


def _build_program():
    import concourse.bass as bass
    import concourse.mybir as mybir
    import concourse.tile as tile
    from concourse import bacc

    f32 = mybir.dt.float32
    bf16 = mybir.dt.bfloat16
    fp8 = mybir.dt.float8e3
    fp8d = mybir.dt.float8e4
    Exp = mybir.ActivationFunctionType.Exp
    Copy = mybir.ActivationFunctionType.Copy

    nc = bacc.Bacc("TRN2", target_bir_lowering=False, debug=False,
                   num_devices=NCORES)

    hsT = nc.declare_dram_parameter("hsT", [D, L], bf16, isOutput=False)
    wqT = nc.declare_dram_parameter("wqT", [D, D], bf16, isOutput=False)
    wkT = nc.declare_dram_parameter("wkT", [D, D], bf16, isOutput=False)
    wvT = nc.declare_dram_parameter("wvT", [D, D], bf16, isOutput=False)
    wkN = nc.declare_dram_parameter("wkN", [D, D], bf16, isOutput=False)
    wqN = nc.declare_dram_parameter("wqN", [D, D], bf16, isOutput=False)
    woT = nc.declare_dram_parameter("woT", [D, D], bf16, isOutput=False)
    # fp8e3 rel streams, partition-contiguous: [group, 128 part, slot, free]
    relS2 = nc.declare_dram_parameter("relS2", [16, 128, 8, L], fp8, isOutput=False)
    relS3 = nc.declare_dram_parameter("relS3", [32, 128, 16, LC], fp8, isOutput=False)
    ident2 = nc.declare_dram_parameter("ident2", [128, 128], bf16, isOutput=False)
    maskc = nc.declare_dram_parameter("maskc", [L, 1], f32, isOutput=False)
    out = nc.declare_dram_parameter("out", [LC, D], f32, isOutput=True)

    s2dump = nc.dram_tensor("s2dump", [16, LC, L], bf16)     # [hslot, l, r]
    s3dumpT = nc.dram_tensor("s3dumpT", [L, 16, LC], fp8d)   # [r, hslot, l]
    zstage = nc.dram_tensor("zstage", [H, LC], mybir.dt.float32)

    NL2 = LC // 2   # 128 l-pairs
    NR2 = L // 2    # 512 r-pairs

    with tile.TileContext(nc) as tc:
        with (
            tc.tile_pool(name="persist", bufs=1) as pp,
            tc.tile_pool(name="work", bufs=3) as kp,
            tc.tile_pool(name="slabs", bufs=1) as sp,
            tc.tile_pool(name="finC", bufs=1) as fp,
        ):
            strm_cm = tc.tile_pool(name="strm", bufs=2)
            strm = strm_cm.__enter__()
            tp_cm = tc.tile_pool(name="tmpP", bufs=1)
            tp = tp_cm.__enter__()
            wl_cm = tc.tile_pool(name="wload", bufs=2)
            wl = wl_cm.__enter__()
            # =================== Phase P ===================
            psp_cm = tc.tile_pool(name="psP", bufs=2, space="PSUM")
            psp = psp_cm.__enter__()

            hsT_t = []
            for et in range(ET):
                t = tp.tile([128, L], bf16, tag=f"hsT{et}")
                nc.sync.dma_start(out=t, in_=hsT[et * 128:(et + 1) * 128, :])
                hsT_t.append(t)

            def load_w(handle):
                t = wl.tile([128, ET, D], bf16, tag="w6", name="w6")
                nc.sync.dma_start(
                    out=t, in_=handle[:, :].rearrange("(et p) e -> p et e", et=ET))
                return [t[:, et, :] for et in range(ET)]

            def load_w_gp(handle):
                t = wl.tile([128, ET, D], bf16, tag="w6", name="w6")
                nc.gpsimd.dma_start(
                    out=t, in_=handle[:, :].rearrange("(et p) e -> p et e", et=ET))
                return [t[:, et, :] for et in range(ET)]

            def copy_ps(dst, src, on_scalar):
                if on_scalar:
                    nc.scalar.activation(dst, src, Copy)
                else:
                    nc.vector.tensor_copy(dst, src)

            wqT_t = load_w(wqT)

            # qT [768, 256] (own chunk = hsT cols 0:256)
            qT_sb = pp.tile([128, ET, LC], bf16, tag="qT")
            for ddt in range(ET):
                ps = psp.tile([128, LC], f32, tag="ps256")
                for et in range(ET):
                    nc.tensor.matmul(
                        ps, wqT_t[et][:, ddt * 128:(ddt + 1) * 128],
                        hsT_t[et][:, 0:LC],
                        start=(et == 0), stop=(et == ET - 1),
                    )
                copy_ps(qT_sb[:, ddt, :], ps, ddt % 2 == 0)

            # rq2: [par*64+dd, hp, j, l] = rq[l, 2j+hp, dd] (both halves dup'd)
            rq2_cm = tc.tile_pool(name="rq2p", bufs=1)
            rq2pool = rq2_cm.__enter__()
            rq2 = rq2pool.tile([128, 2, ET, LC], bf16, tag="rq2")
            wkN_t = load_w(wkN)
            for j in range(ET):
                ps = psp.tile([128, LC], f32, tag="ps256")
                for et in range(ET):
                    nc.tensor.matmul(
                        ps, wkN_t[et][:, j * 128:(j + 1) * 128], qT_sb[:, et, :],
                        start=(et == 0), stop=(et == ET - 1),
                    )
                copy_ps(rq2[0:64, 0, j, :], ps[0:64, :], j % 2 == 0)
                copy_ps(rq2[64:128, 1, j, :], ps[64:128, :], j % 2 == 0)
            nc.gpsimd.dma_start(out=rq2[64:128, 0, :, :], in_=rq2[0:64, 0, :, :])
            nc.gpsimd.dma_start(out=rq2[0:64, 1, :, :], in_=rq2[64:128, 1, :, :])

            # LHS2 block-diagonal slab: col c = par*16 + h (pads zeroed only)
            LHS2 = sp.tile([128, NL2, 32], bf16, tag="LHS2")
            nc.vector.memset(LHS2[:, :, 12:16], 0.0)
            nc.vector.memset(LHS2[:, :, 28:32], 0.0)
            nc.vector.memset(LHS2[0:64, :, 16:28], 0.0)
            nc.vector.memset(LHS2[64:128, :, 0:12], 0.0)
            rq2e = rq2.rearrange("p hp j (lp two) -> p lp j hp two", two=2)
            nc.vector.tensor_copy(
                LHS2[0:64, :, 0:12].rearrange("p lp (j hp) -> p lp j hp", j=6),
                rq2e[0:64, :, :, :, 0],
            )
            nc.vector.tensor_copy(
                LHS2[64:128, :, 16:28].rearrange("p lp (j hp) -> p lp j hp", j=6),
                rq2e[64:128, :, :, :, 1],
            )
            rq2_cm.__exit__(None, None, None)

            # ---------------- P2 chunk emitters (run inside S2 loop) --------
            kT_t = [pp.tile([128, L], bf16, tag=f"kT{ddt}", name=f"kT{ddt}")
                    for ddt in range(ET)]
            V_sb = pp.tile([128, 8, H, d + 1], bf16, tag="V")
            rk2_cm = tc.tile_pool(name="rk2p", bufs=1)
            rk2pool = rk2_cm.__enter__()
            rk2 = rk2pool.tile([128, 2, ET, L], bf16, tag="rk2")
            LHS3 = sp.tile([128, NR2, 32], bf16, tag="LHS3")
            wkT_t = load_w(wkT)
            wvT_t = load_w(wvT)
            state = {}

            def p2_kT(args):
                ddt, nn = args
                ps = psp.tile([128, 512], f32, tag="ps512")
                for et in range(ET):
                    nc.tensor.matmul(
                        ps, wkT_t[et][:, ddt * 128:(ddt + 1) * 128],
                        hsT_t[et][:, nn * 512:(nn + 1) * 512],
                        start=(et == 0), stop=(et == ET - 1),
                    )
                nc.vector.tensor_copy(kT_t[ddt][:, nn * 512:(nn + 1) * 512], ps)

            def p2_vones(args):
                nc.vector.memset(V_sb[:, :, :, d:d + 1], 1.0)

            def p2_V(args):
                rt = args
                psA = psp.tile([128, 512], f32, tag="ps512")
                psB = psp.tile([128, 256], f32, tag="ps256")
                for et in range(ET):
                    lw = hsT_t[et][:, rt * 128:(rt + 1) * 128]
                    nc.tensor.matmul(psA, lw, wvT_t[et][:, 0:512],
                                     start=(et == 0), stop=(et == ET - 1))
                    nc.tensor.matmul(psB, lw, wvT_t[et][:, 512:768],
                                     start=(et == 0), stop=(et == ET - 1))
                nc.vector.tensor_copy(
                    V_sb[:, rt, 0:8, 0:d], psA.rearrange("p (h v) -> p h v", h=8))
                nc.vector.tensor_copy(
                    V_sb[:, rt, 8:12, 0:d], psB.rearrange("p (h v) -> p h v", h=4))

            def p2_wqN(args):
                state["wqN_t"] = load_w_gp(wqN)

            def p2_rk2(args):
                j, nn = args
                wqN_t = state["wqN_t"]
                ps2 = psp.tile([128, 512], f32, tag="ps512")
                for et in range(ET):
                    nc.tensor.matmul(
                        ps2, wqN_t[et][:, j * 128:(j + 1) * 128],
                        kT_t[et][:, nn * 512:(nn + 1) * 512],
                        start=(et == 0), stop=(et == ET - 1),
                    )
                nc.vector.tensor_copy(
                    rk2[0:64, 0, j, nn * 512:(nn + 1) * 512], ps2[0:64, :])
                nc.vector.tensor_copy(
                    rk2[64:128, 1, j, nn * 512:(nn + 1) * 512], ps2[64:128, :])

            def p2_rk2dup(args):
                nc.gpsimd.dma_start(out=rk2[64:128, 0, :, :], in_=rk2[0:64, 0, :, :])
                nc.gpsimd.dma_start(out=rk2[0:64, 1, :, :], in_=rk2[64:128, 1, :, :])

            def p2_lhs3(args):
                half = args
                if half == 0:
                    nc.vector.memset(LHS3[:, :, 12:16], 0.0)
                    nc.vector.memset(LHS3[:, :, 28:32], 0.0)
                    state["rk2e"] = rk2.rearrange(
                        "p hp j (rp two) -> p rp j hp two", two=2)
                    nc.vector.memset(LHS3[0:64, :, 16:28], 0.0)
                    nc.vector.tensor_copy(
                        LHS3[0:64, :, 0:12].rearrange(
                            "p rp (j hp) -> p rp j hp", j=6),
                        state["rk2e"][0:64, :, :, :, 0],
                    )
                else:
                    nc.vector.memset(LHS3[64:128, :, 0:12], 0.0)
                    nc.vector.tensor_copy(
                        LHS3[64:128, :, 16:28].rearrange(
                            "p rp (j hp) -> p rp j hp", j=6),
                        state["rk2e"][64:128, :, :, :, 1],
                    )

            p2_chunks = (
                [(p2_vones, None)]
                + [(p2_kT, (ddt, nn)) for ddt in range(ET) for nn in range(2)]
                + [(p2_wqN, None)]
                + [(p2_V, rt) for rt in range(8)]
                + [(p2_rk2, (j, nn)) for j in range(ET) for nn in range(2)]
                + [(p2_rk2dup, None), (p2_lhs3, 0), (p2_lhs3, 1)]
            )

            # C-phase constants (loaded during P; rings are free)
            mask_sb = fp.tile([128, 8], f32, tag="mask")
            nc.sync.dma_start(
                out=mask_sb, in_=maskc.rearrange("(t p) o -> p (t o)", p=128))
            I2 = fp.tile([128, 128], bf16, tag="I2")
            nc.sync.dma_start(out=I2, in_=ident2[:, :])
            acc = fp.tile([128, H, LC], f32, tag="acc")
            nc.vector.memset(acc[0:65, :, :], 0.0)

            # =================== Phase S2 || P2 ===================
            # dump dst: partition (s4, par, hs) -> [hs, l=8g+2*s4+par, r]
            s2dump_v = s2dump.rearrange("hs (g s p) r -> g s p hs r", s=4, p=2)
            pss_cm = tc.tile_pool(name="psS", bufs=2, space="PSUM")
            pss = pss_cm.__enter__()
            ci = 0
            for gg in range(NL2 // 8):
                stream8 = strm.tile([128, 8, L], fp8, tag="s2stream")
                nc.sync.dma_start(out=stream8, in_=relS2[gg])
                for gh in range(2):
                    g = gg * 2 + gh
                    ps4 = pss.tile([128, L], f32, tag="ps1024")
                    for s4 in range(4):
                        lp = g * 4 + s4
                        for nn in range(2):
                            nc.tensor.matmul(
                                ps4[s4 * 32:(s4 + 1) * 32, nn * 512:(nn + 1) * 512],
                                LHS2[:, lp, :],
                                stream8[:, gh * 4 + s4, nn * 512:(nn + 1) * 512],
                                start=True, stop=True, tile_position=(0, 32 * s4),
                            )
                    cp = kp.tile([128, L], bf16, tag="dumpc2")
                    nc.vector.tensor_copy(cp, ps4)
                    nc.scalar.dma_start(out=s2dump_v[g], in_=cp)
                    # interleave P2 chunks to fill PE gaps (keeps HAM warm)
                    left = len(p2_chunks) - ci
                    slots = 2 * (NL2 // 8) - (2 * gg + gh)
                    take = -(-left // slots) if slots > 0 else left
                    for _ in range(take):
                        if ci < len(p2_chunks):
                            fn, args = p2_chunks[ci]
                            fn(args)
                            ci += 1
            while ci < len(p2_chunks):
                fn, args = p2_chunks[ci]
                fn(args)
                ci += 1
            pss_cm.__exit__(None, None, None)
            rk2_cm.__exit__(None, None, None)
            psp_cm.__exit__(None, None, None)
            wl_cm.__exit__(None, None, None)
            tp_cm.__exit__(None, None, None)

            # =================== Phase S3 || C (per r-tile) ===================
            s3dumpT_v = s3dumpT.rearrange("(G g s p) hs l -> G g (s p hs) l",
                                          g=4, s=4, p=2)
            ps3_cm = tc.tile_pool(name="psS3", bufs=2, space="PSUM")
            ps3 = ps3_cm.__enter__()
            scr_cm = tc.tile_pool(name="scores", bufs=2, space="PSUM")
            scr = scr_cm.__enter__()
            pvp_cm = tc.tile_pool(name="pvacc", bufs=2, space="PSUM")
            pvp = pvp_cm.__enter__()
            rl_cm = tc.tile_pool(name="rload", bufs=2)
            rl = rl_cm.__enter__()

            for rt in range(8):
                for G in range(rt * 4, rt * 4 + 4):
                    stream16 = strm.tile([128, 16, LC], fp8, tag="s3stream")
                    nc.sync.dma_start(out=stream16, in_=relS3[G])
                    cp4 = kp.tile([128, 4, LC], fp8d, tag="dumpc3")
                    for gp in range(2):
                        ps4 = ps3.tile([128, 512], f32, tag="s3512", name="ps4")
                        for g4h in range(2):
                            g = G * 4 + 2 * gp + g4h
                            for s4 in range(4):
                                rp = g * 4 + s4
                                nc.tensor.matmul(
                                    ps4[s4 * 32:(s4 + 1) * 32,
                                        g4h * 256:(g4h + 1) * 256],
                                    LHS3[:, rp, :],
                                    stream16[:, (2 * gp + g4h) * 4 + s4, :],
                                    start=True, stop=True,
                                    tile_position=(0, 32 * s4),
                                )
                        nc.vector.tensor_copy(
                            cp4[:, 2 * gp:2 * gp + 2, :].rearrange(
                                "p g l -> p (g l)"),
                            ps4)
                    nc.sync.dma_start(
                        out=s3dumpT_v[G].rearrange("g p l -> p g l"), in_=cp4)

                s3r = rl.tile([128, H, LC], fp8d, tag="s3r")
                nc.scalar.dma_start(
                    out=s3r, in_=s3dumpT[rt * 128:(rt + 1) * 128, 0:H, :])
                s2r = rl.tile([128, H, LC], bf16, tag="s2r")
                nc.sync.dma_start_transpose(
                    out=s2r.rearrange("p hs l -> p (hs l)"),
                    in_=s2dump.rearrange("hs l r -> (hs l) r")[
                        0:H * LC, rt * 128:(rt + 1) * 128],
                )
                for h in range(H):
                    hp = h % 2
                    ps = scr.tile([128, LC], f32, tag="sc256", name="ps")
                    nc.tensor.matmul(
                        ps,
                        kT_t[h // 2][hp * 64:(hp + 1) * 64,
                                     rt * 128:(rt + 1) * 128],
                        qT_sb[hp * 64:(hp + 1) * 64, h // 2, :],
                        start=True, stop=False, tile_position=(hp * 64, 0),
                    )
                    nc.tensor.matmul(
                        ps, I2, s2r[:, h, :],
                        start=False, stop=False, tile_position=(0, 0),
                    )
                    nc.tensor.matmul(
                        ps, I2, s3r[:, h, :],
                        start=False, stop=True, tile_position=(0, 0),
                    )
                    probs = kp.tile([128, LC], bf16, tag="probs")
                    nc.scalar.activation(
                        probs, ps, Exp,
                        bias=mask_sb[:, rt:rt + 1], scale=SCALE,
                    )
                    pv = pvp.tile([128, LC], f32, tag="pvps", name="pv")
                    nc.tensor.matmul(
                        pv[0:65, :], V_sb[:, rt, h, :], probs,
                        start=True, stop=True, tile_position=(0, 0),
                    )
                    nc.vector.tensor_add(
                        acc[0:65, h, :], acc[0:65, h, :], pv[0:65, :])

            rl_cm.__exit__(None, None, None)
            pvp_cm.__exit__(None, None, None)
            scr_cm.__exit__(None, None, None)
            ps3_cm.__exit__(None, None, None)
            strm_cm.__exit__(None, None, None)

            # ---------------- tail: normalize + output projection ----------
            woT12 = fp.tile([64, H, D], bf16, tag="woT12")
            nc.sync.dma_start(
                out=woT12, in_=woT.rearrange("(h p) e -> p h e", h=H))
            zb = fp.tile([64, H, LC], f32, tag="zb")
            nc.scalar.dma_start(out=zstage[:, :], in_=acc[64:65, :, :])
            zs = zstage[:, :]
            zb_src = bass.AP(
                tensor=zs.tensor, offset=zs.offset,
                ap=[[0, 64]] + [list(x) for x in zs.ap],
            )
            nc.gpsimd.dma_start(out=zb, in_=zb_src)
            nc.vector.reciprocal(zb, zb)
            att12 = fp.tile([64, H, LC], bf16, tag="att12")
            nc.vector.tensor_mul(att12, acc[0:64, :, :], zb)

            with tc.tile_pool(name="psO", bufs=2, space="PSUM") as pso:
                for lh in range(2):
                    psA = pso.tile([128, 512], f32, tag="oA")
                    psB = pso.tile([128, 256], f32, tag="oB")
                    for h in range(H):
                        lw = att12[:, h, lh * 128:(lh + 1) * 128]
                        nc.tensor.matmul(psA, lw, woT12[:, h, 0:512],
                                         start=(h == 0), stop=(h == H - 1))
                        nc.tensor.matmul(psB, lw, woT12[:, h, 512:768],
                                         start=(h == 0), stop=(h == H - 1))
                    osb = kp.tile([128, D], f32, tag="osb", bufs=1)
                    nc.vector.tensor_copy(osb[:, 0:512], psA)
                    nc.vector.tensor_copy(osb[:, 512:768], psB)
                    nc.scalar.dma_start(
                        out=out[lh * 128:(lh + 1) * 128, :], in_=osb)

    nc.compile()
    return nc


def _get_program():
    if "nc" not in _PROG_CACHE:
        _PROG_CACHE["nc"] = _build_program()
    return _PROG_CACHE["nc"]


def _host_prep(inputs):
    hs = np.asarray(inputs["hidden_states"], np.float32)
    mask = np.asarray(inputs["attention_mask"], np.float32)
    rel = np.asarray(inputs["relative_attentions"], np.float32)
    Wq = np.asarray(inputs["Wq"], np.float32)
    Wk = np.asarray(inputs["Wk"], np.float32)
    Wv = np.asarray(inputs["Wv"], np.float32)
    Wo = np.asarray(inputs["Wo"], np.float32)

    wqT = np.ascontiguousarray(Wq.T).astype(BF16)
    wkT = np.ascontiguousarray(Wk.T).astype(BF16)
    wvT = np.ascontiguousarray(Wv.T).astype(BF16)
    # halved so S2/S3 stay in fp8 range; phase C re-doubles via 2*I
    wkN = np.ascontiguousarray(Wk * 0.5).astype(BF16)
    wqN = np.ascontiguousarray(Wq * 0.5).astype(BF16)
    woT = np.ascontiguousarray(Wo.T).astype(BF16)
    ident2 = (np.eye(128, dtype=np.float32) * 2.0).astype(BF16)

    in_maps = []
    for core in range(NCORES):
        b, lci = divmod(core, 4)
        lo = lci * LC
        hs_roll = np.roll(hs[b], -lo, axis=0)
        hsT_np = np.ascontiguousarray(hs_roll.T).astype(BF16)
        rel_c = np.roll(rel[b, lo:lo + LC], -lo, axis=1)   # [256 l, 1024 r, 64 d]
        rel8 = rel_c.astype(FP8)
        # [l, r, d] -> [lp, (l2 d), r] -> [gg=16, 128, s=8, r=1024]
        relS2_np = np.ascontiguousarray(
            rel8.transpose(0, 2, 1).reshape(128, 128, L)
            .reshape(16, 8, 128, L).transpose(0, 2, 1, 3))
        # [l, r, d] -> [rp, (r2 d), l] -> [G=32, 128, s=16, l=256]
        relS3_np = np.ascontiguousarray(
            rel8.transpose(1, 2, 0).reshape(512, 128, LC)
            .reshape(32, 16, 128, LC).transpose(0, 2, 1, 3))
        maskc_np = np.ascontiguousarray(
            np.roll(mask[b, 0, 0, :], -lo).reshape(L, 1))
        in_maps.append({
            "hsT": hsT_np, "wqT": wqT, "wkT": wkT, "wvT": wvT,
            "wkN": wkN, "wqN": wqN, "woT": woT, "ident2": ident2,
            "relS2": relS2_np, "relS3": relS3_np, "maskc": maskc_np,
        })
    return in_maps


def _host_post(inputs, results):
    out = np.empty((B, L, D), np.float32)
    for core in range(NCORES):
        b, lci = divmod(core, 4)
        out[b, lci * LC:(lci + 1) * LC, :] = results[core]["out"]
    bo2 = (np.asarray(inputs["Wo"], np.float32) @ np.asarray(inputs["bv"], np.float32)
           + np.asarray(inputs["bo"], np.float32))
    out += bo2[None, None, :]
    return out


def kernel(**inputs):
    from concourse.bass_utils import run_bass_kernel_spmd

    nc = _get_program()
    in_maps = _host_prep(inputs)
    res = run_bass_kernel_spmd(nc, in_maps, list(range(NCORES)))
    return _host_post(inputs, res.results)


# revision 44
# speedup vs baseline: 1.0205x; 1.0205x over previous
"""MoTAttention Trainium2 kernel (self-contained).

B,L,D,H = 2,1024,768,12 ; d=64.  8 cores = (batch b in {0,1}) x (4 l-chunks of 256).
SPMD: one program; per-core data differs. Each core's hidden_states are ROLLED so
its own l-chunk sits at rows 0:256; rel's r-axis and the mask are rolled
identically (softmax/PV are invariant to a consistent permutation of r).

Phased structure (phases keep big-packet and small-packet DMA flows apart,
which the SDMA engines reward):
  P:  projections qT, rq -> LHS2; kT, V+ones, rk -> LHS3 (PSUM->SBUF copies
      split between the Vector and Scalar engines).
  S2: S2[h,l,r] l-pair matmuls (K=128 = 2l x 64d, M=32, 4 pairs/PSUM via
      col-group tile_position), rel streamed fp8e3 as 1MB partition-contiguous
      super-groups; dump bf16 [hs, l, r] (transposed reload needs 2B dtype).
  S3: same per r-pair, 2 dump-groups per PSUM bank; dump fp8e4 to
      s3dumpT == [r, hs, l] (phase-C reads are partition-contiguous 3KB rows).
  C:  per r-tile of 128: DMA-transpose reload of S2 (-> [r, (h,l)]) on the
      sync ring (idle in this phase), plain reload of S3; per head PSUM =
      S1 (q.kT head-sliced matmul) + 2I@s2r + 2I@s3r (tensor-engine adds;
      wkN/wqN are halved on host so S2/S3 fit fp8 range, the 2*I re-doubles);
      probs = exp(PSUM*SCALE + mask_r) straight off PSUM; PV with ones-column
      accumulates unnormalized attT + Z into SBUF.
  Tail: normalize via broadcast + partition-parallel reciprocal; output
      projection.
Host: layout prep only (casts/rolls/transposes) + exact bias fold
      (out += Wo@bv + bo; valid because softmax rows sum to 1).
"""

import math
import numpy as np
import ml_dtypes

BF16 = ml_dtypes.bfloat16
FP8 = ml_dtypes.float8_e3m4
B, L, D, H = 2, 1024, 768, 12
d = 64
LC = 256
NCORES = 8
ET = D // 128
SCALE = 1.0 / math.sqrt(3 * d)

_PROG_CACHE = {}


def _build_program():
    import concourse.bass as bass
    import concourse.mybir as mybir
    import concourse.tile as tile
    from concourse import bacc

    f32 = mybir.dt.float32
    bf16 = mybir.dt.bfloat16
    fp8 = mybir.dt.float8e3
    fp8d = mybir.dt.float8e4
    Exp = mybir.ActivationFunctionType.Exp
    Copy = mybir.ActivationFunctionType.Copy

    nc = bacc.Bacc("TRN2", target_bir_lowering=False, debug=False,
                   num_devices=NCORES)

    hsT = nc.declare_dram_parameter("hsT", [D, L], bf16, isOutput=False)
    wqT = nc.declare_dram_parameter("wqT", [D, D], bf16, isOutput=False)
    wkT = nc.declare_dram_parameter("wkT", [D, D], bf16, isOutput=False)
    wvT = nc.declare_dram_parameter("wvT", [D, D], bf16, isOutput=False)
    wkN = nc.declare_dram_parameter("wkN", [D, D], bf16, isOutput=False)
    wqN = nc.declare_dram_parameter("wqN", [D, D], bf16, isOutput=False)
    woT = nc.declare_dram_parameter("woT", [D, D], bf16, isOutput=False)
    # fp8e3 rel streams, partition-contiguous: [group, 128 part, slot, free]
    relS2 = nc.declare_dram_parameter("relS2", [16, 128, 8, L], fp8, isOutput=False)
    relS3 = nc.declare_dram_parameter("relS3", [32, 128, 16, LC], fp8, isOutput=False)
    ident2 = nc.declare_dram_parameter("ident2", [128, 128], bf16, isOutput=False)
    maskc = nc.declare_dram_parameter("maskc", [L, 1], f32, isOutput=False)
    out = nc.declare_dram_parameter("out", [LC, D], f32, isOutput=True)

    s2dump = nc.dram_tensor("s2dump", [16, LC, L], bf16)     # [hslot, l, r]
    s3dumpT = nc.dram_tensor("s3dumpT", [L, 16, LC], fp8d)   # [r, hslot, l]
    zstage = nc.dram_tensor("zstage", [H, LC], mybir.dt.float32)

    NL2 = LC // 2   # 128 l-pairs
    NR2 = L // 2    # 512 r-pairs

    with tile.TileContext(nc) as tc:
        with (
            tc.tile_pool(name="persist", bufs=1) as pp,
            tc.tile_pool(name="work", bufs=3) as kp,
            tc.tile_pool(name="slabs", bufs=1) as sp,
            tc.tile_pool(name="finC", bufs=1) as fp,
        ):
            strm_cm = tc.tile_pool(name="strm", bufs=2)
            strm = strm_cm.__enter__()
            tp_cm = tc.tile_pool(name="tmpP", bufs=1)
            tp = tp_cm.__enter__()
            wl_cm = tc.tile_pool(name="wload", bufs=2)
            wl = wl_cm.__enter__()
            # =================== Phase P ===================
            psp_cm = tc.tile_pool(name="psP", bufs=2, space="PSUM")
            psp = psp_cm.__enter__()

            hsT_t = []
            for et in range(ET):
                t = tp.tile([128, L], bf16, tag=f"hsT{et}")
                nc.sync.dma_start(out=t, in_=hsT[et * 128:(et + 1) * 128, :])
                hsT_t.append(t)

            def load_w(handle):
                t = wl.tile([128, ET, D], bf16, tag="w6", name="w6")
                nc.sync.dma_start(
                    out=t, in_=handle[:, :].rearrange("(et p) e -> p et e", et=ET))
                return [t[:, et, :] for et in range(ET)]

            def copy_ps(dst, src, on_scalar):
                if on_scalar:
                    nc.scalar.activation(dst, src, Copy)
                else:
                    nc.vector.tensor_copy(dst, src)

            wqT_t = load_w(wqT)

            # qT [768, 256] (own chunk = hsT cols 0:256)
            qT_sb = pp.tile([128, ET, LC], bf16, tag="qT")
            for ddt in range(ET):
                ps = psp.tile([128, LC], f32, tag="ps256")
                for et in range(ET):
                    nc.tensor.matmul(
                        ps, wqT_t[et][:, ddt * 128:(ddt + 1) * 128],
                        hsT_t[et][:, 0:LC],
                        start=(et == 0), stop=(et == ET - 1),
                    )
                copy_ps(qT_sb[:, ddt, :], ps, ddt % 2 == 0)

            # rq2: [par*64+dd, hp, j, l] = rq[l, 2j+hp, dd] (both halves dup'd)
            rq2_cm = tc.tile_pool(name="rq2p", bufs=1)
            rq2pool = rq2_cm.__enter__()
            rq2 = rq2pool.tile([128, 2, ET, LC], bf16, tag="rq2")
            wkN_t = load_w(wkN)
            for j in range(ET):
                ps = psp.tile([128, LC], f32, tag="ps256")
                for et in range(ET):
                    nc.tensor.matmul(
                        ps, wkN_t[et][:, j * 128:(j + 1) * 128], qT_sb[:, et, :],
                        start=(et == 0), stop=(et == ET - 1),
                    )
                copy_ps(rq2[0:64, 0, j, :], ps[0:64, :], j % 2 == 0)
                copy_ps(rq2[64:128, 1, j, :], ps[64:128, :], j % 2 == 0)
            nc.gpsimd.dma_start(out=rq2[64:128, 0, :, :], in_=rq2[0:64, 0, :, :])
            nc.gpsimd.dma_start(out=rq2[0:64, 1, :, :], in_=rq2[64:128, 1, :, :])

            # LHS2 block-diagonal slab: col c = par*16 + h (pads zeroed only)
            LHS2 = sp.tile([128, NL2, 32], bf16, tag="LHS2")
            nc.vector.memset(LHS2[:, :, 12:16], 0.0)
            nc.vector.memset(LHS2[:, :, 28:32], 0.0)
            nc.vector.memset(LHS2[0:64, :, 16:28], 0.0)
            nc.vector.memset(LHS2[64:128, :, 0:12], 0.0)
            rq2e = rq2.rearrange("p hp j (lp two) -> p lp j hp two", two=2)
            nc.vector.tensor_copy(
                LHS2[0:64, :, 0:12].rearrange("p lp (j hp) -> p lp j hp", j=6),
                rq2e[0:64, :, :, :, 0],
            )
            nc.vector.tensor_copy(
                LHS2[64:128, :, 16:28].rearrange("p lp (j hp) -> p lp j hp", j=6),
                rq2e[64:128, :, :, :, 1],
            )
            rq2_cm.__exit__(None, None, None)

            # kT [768, 1024]
            kT_t = [pp.tile([128, L], bf16, tag=f"kT{ddt}", name=f"kT{ddt}")
                    for ddt in range(ET)]
            wkT_t = load_w(wkT)
            for ddt in range(ET):
                for nn in range(2):
                    ps = psp.tile([128, 512], f32, tag="ps512")
                    for et in range(ET):
                        nc.tensor.matmul(
                            ps, wkT_t[et][:, ddt * 128:(ddt + 1) * 128],
                            hsT_t[et][:, nn * 512:(nn + 1) * 512],
                            start=(et == 0), stop=(et == ET - 1),
                        )
                    copy_ps(kT_t[ddt][:, nn * 512:(nn + 1) * 512], ps, nn == 0)

            # v natural + ones col: V_sb [128, 8, 12, 65]
            wvT_t = load_w(wvT)
            V_sb = pp.tile([128, 8, H, d + 1], bf16, tag="V")
            nc.vector.memset(V_sb[:, :, :, d:d + 1], 1.0)
            for rt in range(8):
                psA = psp.tile([128, 512], f32, tag="ps512")
                psB = psp.tile([128, 256], f32, tag="ps256")
                for et in range(ET):
                    lw = hsT_t[et][:, rt * 128:(rt + 1) * 128]
                    nc.tensor.matmul(psA, lw, wvT_t[et][:, 0:512],
                                     start=(et == 0), stop=(et == ET - 1))
                    nc.tensor.matmul(psB, lw, wvT_t[et][:, 512:768],
                                     start=(et == 0), stop=(et == ET - 1))
                copy_ps(V_sb[:, rt, 0:8, 0:d],
                        psA.rearrange("p (h v) -> p h v", h=8), rt % 2 == 0)
                copy_ps(V_sb[:, rt, 8:12, 0:d],
                        psB.rearrange("p (h v) -> p h v", h=4), rt % 2 == 0)

            # rk2 + LHS3
            rk2_cm = tc.tile_pool(name="rk2p", bufs=1)
            rk2pool = rk2_cm.__enter__()
            rk2 = rk2pool.tile([128, 2, ET, L], bf16, tag="rk2")
            wqN_t = load_w(wqN)
            for j in range(ET):
                for nn in range(2):
                    ps2 = psp.tile([128, 512], f32, tag="ps512")
                    for et in range(ET):
                        nc.tensor.matmul(
                            ps2, wqN_t[et][:, j * 128:(j + 1) * 128],
                            kT_t[et][:, nn * 512:(nn + 1) * 512],
                            start=(et == 0), stop=(et == ET - 1),
                        )
                    copy_ps(rk2[0:64, 0, j, nn * 512:(nn + 1) * 512],
                            ps2[0:64, :], nn == 0)
                    copy_ps(rk2[64:128, 1, j, nn * 512:(nn + 1) * 512],
                            ps2[64:128, :], nn == 0)
            nc.gpsimd.dma_start(out=rk2[64:128, 0, :, :], in_=rk2[0:64, 0, :, :])
            nc.gpsimd.dma_start(out=rk2[0:64, 1, :, :], in_=rk2[64:128, 1, :, :])

            LHS3 = sp.tile([128, NR2, 32], bf16, tag="LHS3")
            nc.vector.memset(LHS3[:, :, 12:16], 0.0)
            nc.vector.memset(LHS3[:, :, 28:32], 0.0)
            nc.vector.memset(LHS3[0:64, :, 16:28], 0.0)
            nc.vector.memset(LHS3[64:128, :, 0:12], 0.0)
            rk2e = rk2.rearrange("p hp j (rp two) -> p rp j hp two", two=2)
            nc.vector.tensor_copy(
                LHS3[0:64, :, 0:12].rearrange("p rp (j hp) -> p rp j hp", j=6),
                rk2e[0:64, :, :, :, 0],
            )
            nc.vector.tensor_copy(
                LHS3[64:128, :, 16:28].rearrange("p rp (j hp) -> p rp j hp", j=6),
                rk2e[64:128, :, :, :, 1],
            )

            rk2_cm.__exit__(None, None, None)

            # C-phase constants (loaded during P; sync ring is free)
            mask_sb = fp.tile([128, 8], f32, tag="mask")
            nc.sync.dma_start(
                out=mask_sb, in_=maskc.rearrange("(t p) o -> p (t o)", p=128))
            I2 = fp.tile([128, 128], bf16, tag="I2")
            nc.sync.dma_start(out=I2, in_=ident2[:, :])
            acc = fp.tile([128, H, LC], f32, tag="acc")
            nc.vector.memset(acc[0:65, :, :], 0.0)

            # =================== Phase S2 ===================
            # dump dst: partition (s4, par, hs) -> [hs, l=8g+2*s4+par, r]
            s2dump_v = s2dump.rearrange("hs (g s p) r -> g s p hs r", s=4, p=2)
            pss_cm = tc.tile_pool(name="psS", bufs=2, space="PSUM")
            pss = pss_cm.__enter__()
            for gg in range(NL2 // 8):
                stream8 = strm.tile([128, 8, L], fp8, tag="s2stream")
                nc.sync.dma_start(out=stream8, in_=relS2[gg])
                for gh in range(2):
                    g = gg * 2 + gh
                    ps4 = pss.tile([128, L], f32, tag="ps1024")
                    for s4 in range(4):
                        lp = g * 4 + s4
                        for nn in range(2):
                            nc.tensor.matmul(
                                ps4[s4 * 32:(s4 + 1) * 32, nn * 512:(nn + 1) * 512],
                                LHS2[:, lp, :],
                                stream8[:, gh * 4 + s4, nn * 512:(nn + 1) * 512],
                                start=True, stop=True, tile_position=(0, 32 * s4),
                            )
                    cp = kp.tile([128, L], bf16, tag="dumpc2")
                    nc.vector.tensor_copy(cp, ps4)
                    nc.scalar.dma_start(out=s2dump_v[g], in_=cp)
            pss_cm.__exit__(None, None, None)
            psp_cm.__exit__(None, None, None)
            wl_cm.__exit__(None, None, None)
            tp_cm.__exit__(None, None, None)

            # =================== Phase S3 ===================
            s3dumpT_v = s3dumpT.rearrange("(G g s p) hs l -> G g (s p hs) l",
                                          g=4, s=4, p=2)
            scr_cm = tc.tile_pool(name="scores", bufs=4, space="PSUM")
            scr = scr_cm.__enter__()
            pvp_cm = tc.tile_pool(name="pvacc", bufs=3, space="PSUM")
            pvp = pvp_cm.__enter__()
            rl_cm = tc.tile_pool(name="rload", bufs=2)
            rl = rl_cm.__enter__()

            for G in range(NR2 // 16):
                stream16 = strm.tile([128, 16, LC], fp8, tag="s3stream")
                nc.sync.dma_start(out=stream16, in_=relS3[G])
                cp4 = kp.tile([128, 4, LC], fp8d, tag="dumpc3")
                for gp in range(2):
                    ps4 = scr.tile([128, 512], f32, tag="sc512", name="ps4")
                    for g4h in range(2):
                        g = G * 4 + 2 * gp + g4h
                        for s4 in range(4):
                            rp = g * 4 + s4
                            nc.tensor.matmul(
                                ps4[s4 * 32:(s4 + 1) * 32,
                                    g4h * 256:(g4h + 1) * 256],
                                LHS3[:, rp, :],
                                stream16[:, (2 * gp + g4h) * 4 + s4, :],
                                start=True, stop=True,
                                tile_position=(0, 32 * s4),
                            )
                    nc.vector.tensor_copy(
                        cp4[:, 2 * gp:2 * gp + 2, :].rearrange("p g l -> p (g l)"),
                        ps4)
                nc.scalar.dma_start(
                    out=s3dumpT_v[G].rearrange("g p l -> p g l"), in_=cp4)

            # =================== Phase C ===================
            for rt in range(8):
                s3r = rl.tile([128, H, LC], fp8d, tag="s3r")
                nc.scalar.dma_start(
                    out=s3r, in_=s3dumpT[rt * 128:(rt + 1) * 128, 0:H, :])
                s2r = rl.tile([128, H, LC], bf16, tag="s2r")
                nc.sync.dma_start_transpose(
                    out=s2r.rearrange("p hs l -> p (hs l)"),
                    in_=s2dump.rearrange("hs l r -> (hs l) r")[
                        0:H * LC, rt * 128:(rt + 1) * 128],
                )
                for h in range(H):
                    hp = h % 2
                    ps = scr.tile([128, 512], f32, tag="sc512", name="ps")[:, 0:LC]
                    nc.tensor.matmul(
                        ps,
                        kT_t[h // 2][hp * 64:(hp + 1) * 64,
                                     rt * 128:(rt + 1) * 128],
                        qT_sb[hp * 64:(hp + 1) * 64, h // 2, :],
                        start=True, stop=False, tile_position=(hp * 64, 0),
                    )
                    nc.tensor.matmul(
                        ps, I2, s2r[:, h, :],
                        start=False, stop=False, tile_position=(0, 0),
                    )
                    nc.tensor.matmul(
                        ps, I2, s3r[:, h, :],
                        start=False, stop=True, tile_position=(0, 0),
                    )
                    probs = kp.tile([128, LC], bf16, tag="probs", bufs=4)
                    nc.scalar.activation(
                        probs, ps, Exp,
                        bias=mask_sb[:, rt:rt + 1], scale=SCALE,
                    )
                    pv = pvp.tile([128, LC], f32, tag="pvps", name="pv")
                    nc.tensor.matmul(
                        pv[0:65, :], V_sb[:, rt, h, :], probs,
                        start=True, stop=True, tile_position=(0, 0),
                    )
                    nc.vector.tensor_add(
                        acc[0:65, h, :], acc[0:65, h, :], pv[0:65, :])

            rl_cm.__exit__(None, None, None)
            pvp_cm.__exit__(None, None, None)
            scr_cm.__exit__(None, None, None)
            strm_cm.__exit__(None, None, None)

            # ---------------- tail: normalize + output projection ----------
            woT12 = fp.tile([64, H, D], bf16, tag="woT12")
            nc.sync.dma_start(
                out=woT12, in_=woT.rearrange("(h p) e -> p h e", h=H))
            zb = fp.tile([64, H, LC], f32, tag="zb")
            nc.scalar.dma_start(out=zstage[:, :], in_=acc[64:65, :, :])
            zs = zstage[:, :]
            zb_src = bass.AP(
                tensor=zs.tensor, offset=zs.offset,
                ap=[[0, 64]] + [list(x) for x in zs.ap],
            )
            nc.gpsimd.dma_start(out=zb, in_=zb_src)
            nc.vector.reciprocal(zb, zb)
            att12 = fp.tile([64, H, LC], bf16, tag="att12")
            nc.vector.tensor_mul(att12, acc[0:64, :, :], zb)

            with tc.tile_pool(name="psO", bufs=2, space="PSUM") as pso:
                for lh in range(2):
                    psA = pso.tile([128, 512], f32, tag="oA")
                    psB = pso.tile([128, 256], f32, tag="oB")
                    for h in range(H):
                        lw = att12[:, h, lh * 128:(lh + 1) * 128]
                        nc.tensor.matmul(psA, lw, woT12[:, h, 0:512],
                                         start=(h == 0), stop=(h == H - 1))
                        nc.tensor.matmul(psB, lw, woT12[:, h, 512:768],
                                         start=(h == 0), stop=(h == H - 1))
                    osb = kp.tile([128, D], f32, tag="osb", bufs=1)
                    nc.vector.tensor_copy(osb[:, 0:512], psA)
                    nc.vector.tensor_copy(osb[:, 512:768], psB)
                    nc.scalar.dma_start(
                        out=out[lh * 128:(lh + 1) * 128, :], in_=osb)

    nc.compile()
    return nc


def _get_program():
    if "nc" not in _PROG_CACHE:
        _PROG_CACHE["nc"] = _build_program()
    return _PROG_CACHE["nc"]


def _host_prep(inputs):
    hs = np.asarray(inputs["hidden_states"], np.float32)
    mask = np.asarray(inputs["attention_mask"], np.float32)
    rel = np.asarray(inputs["relative_attentions"], np.float32)
    Wq = np.asarray(inputs["Wq"], np.float32)
    Wk = np.asarray(inputs["Wk"], np.float32)
    Wv = np.asarray(inputs["Wv"], np.float32)
    Wo = np.asarray(inputs["Wo"], np.float32)

    wqT = np.ascontiguousarray(Wq.T).astype(BF16)
    wkT = np.ascontiguousarray(Wk.T).astype(BF16)
    wvT = np.ascontiguousarray(Wv.T).astype(BF16)
    # halved so S2/S3 stay in fp8 range; phase C re-doubles via 2*I
    wkN = np.ascontiguousarray(Wk * 0.5).astype(BF16)
    wqN = np.ascontiguousarray(Wq * 0.5).astype(BF16)
    woT = np.ascontiguousarray(Wo.T).astype(BF16)
    ident2 = (np.eye(128, dtype=np.float32) * 2.0).astype(BF16)

    in_maps = []
    for core in range(NCORES):
        b, lci = divmod(core, 4)
        lo = lci * LC
        hs_roll = np.roll(hs[b], -lo, axis=0)
        hsT_np = np.ascontiguousarray(hs_roll.T).astype(BF16)
        rel_c = np.roll(rel[b, lo:lo + LC], -lo, axis=1)   # [256 l, 1024 r, 64 d]
        rel8 = rel_c.astype(FP8)
        # [l, r, d] -> [lp, (l2 d), r] -> [gg=16, 128, s=8, r=1024]
        relS2_np = np.ascontiguousarray(
            rel8.transpose(0, 2, 1).reshape(128, 128, L)
            .reshape(16, 8, 128, L).transpose(0, 2, 1, 3))
        # [l, r, d] -> [rp, (r2 d), l] -> [G=32, 128, s=16, l=256]
        relS3_np = np.ascontiguousarray(
            rel8.transpose(1, 2, 0).reshape(512, 128, LC)
            .reshape(32, 16, 128, LC).transpose(0, 2, 1, 3))
        maskc_np = np.ascontiguousarray(
            np.roll(mask[b, 0, 0, :], -lo).reshape(L, 1))
        in_maps.append({
            "hsT": hsT_np, "wqT": wqT, "wkT": wkT, "wvT": wvT,
            "wkN": wkN, "wqN": wqN, "woT": woT, "ident2": ident2,
            "relS2": relS2_np, "relS3": relS3_np, "maskc": maskc_np,
        })
    return in_maps


def _host_post(inputs, results):
    out = np.empty((B, L, D), np.float32)
    for core in range(NCORES):
        b, lci = divmod(core, 4)
        out[b, lci * LC:(lci + 1) * LC, :] = results[core]["out"]
    bo2 = (np.asarray(inputs["Wo"], np.float32) @ np.asarray(inputs["bv"], np.float32)
           + np.asarray(inputs["bo"], np.float32))
    out += bo2[None, None, :]
    return out


def kernel(**inputs):
    from concourse.bass_utils import run_bass_kernel_spmd

    nc = _get_program()
    in_maps = _host_prep(inputs)
    res = run_bass_kernel_spmd(nc, in_maps, list(range(NCORES)))
    return _host_post(inputs, res.results)


# revision 48
# speedup vs baseline: 1.0602x; 1.0389x over previous
"""MoTAttention Trainium2 kernel (self-contained).

B,L,D,H = 2,1024,768,12 ; d=64.  8 cores = (batch b in {0,1}) x (4 l-chunks of 256).
SPMD: one program; per-core data differs. Each core's hidden_states are ROLLED so
its own l-chunk sits at rows 0:256; rel's r-axis and the mask are rolled
identically (softmax/PV are invariant to a consistent permutation of r).

Phased structure (phases keep big-packet and small-packet DMA flows apart,
which the SDMA engines reward):
  P:  projections qT, rq -> LHS2; kT, V+ones, rk -> LHS3 (PSUM->SBUF copies
      split between the Vector and Scalar engines).
  S2: S2[h,l,r] l-pair matmuls (K=128 = 2l x 64d, M=32, 4 pairs/PSUM via
      col-group tile_position), rel streamed fp8e3 as 1MB partition-contiguous
      super-groups; dump bf16 [hs, l, r] (transposed reload needs 2B dtype).
  S3: same per r-pair, 2 dump-groups per PSUM bank; dump fp8e4 to
      s3dumpT == [r, hs, l] (phase-C reads are partition-contiguous 3KB rows).
  C:  per r-tile of 128: DMA-transpose reload of S2 (-> [r, (h,l)]) on the
      sync ring (idle in this phase), plain reload of S3; per head PSUM =
      S1 (q.kT head-sliced matmul) + 2I@s2r + 2I@s3r (tensor-engine adds;
      wkN/wqN are halved on host so S2/S3 fit fp8 range, the 2*I re-doubles);
      probs = exp(PSUM*SCALE + mask_r) straight off PSUM; PV with ones-column
      accumulates unnormalized attT + Z into SBUF.
  Tail: normalize via broadcast + partition-parallel reciprocal; output
      projection.
Host: layout prep only (casts/rolls/transposes) + exact bias fold
      (out += Wo@bv + bo; valid because softmax rows sum to 1).
"""

import math
import numpy as np
import ml_dtypes

BF16 = ml_dtypes.bfloat16
FP8 = ml_dtypes.float8_e3m4
B, L, D, H = 2, 1024, 768, 12
d = 64
LC = 256
NCORES = 8
ET = D // 128
SCALE = 1.0 / math.sqrt(3 * d)

_PROG_CACHE = {}


def _build_program():
    import concourse.bass as bass
    import concourse.mybir as mybir
    import concourse.tile as tile
    from concourse import bacc

    f32 = mybir.dt.float32
    bf16 = mybir.dt.bfloat16
    fp8 = mybir.dt.float8e3
    fp8d = mybir.dt.float8e4
    Exp = mybir.ActivationFunctionType.Exp
    Copy = mybir.ActivationFunctionType.Copy

    nc = bacc.Bacc("TRN2", target_bir_lowering=False, debug=False,
                   num_devices=NCORES)

    hsT = nc.declare_dram_parameter("hsT", [D, L], bf16, isOutput=False)
    wqT = nc.declare_dram_parameter("wqT", [D, D], bf16, isOutput=False)
    wkT = nc.declare_dram_parameter("wkT", [D, D], bf16, isOutput=False)
    wvT = nc.declare_dram_parameter("wvT", [D, D], bf16, isOutput=False)
    wkN = nc.declare_dram_parameter("wkN", [D, D], bf16, isOutput=False)
    wqN = nc.declare_dram_parameter("wqN", [D, D], bf16, isOutput=False)
    woT = nc.declare_dram_parameter("woT", [D, D], bf16, isOutput=False)
    # fp8e3 rel streams, partition-contiguous: [group, 128 part, slot, free]
    relS2 = nc.declare_dram_parameter("relS2", [16, 128, 8, L], fp8, isOutput=False)
    relS3 = nc.declare_dram_parameter("relS3", [32, 128, 16, LC], fp8, isOutput=False)
    ident2 = nc.declare_dram_parameter("ident2", [128, 128], bf16, isOutput=False)
    maskc = nc.declare_dram_parameter("maskc", [L, 1], f32, isOutput=False)
    out = nc.declare_dram_parameter("out", [LC, D], f32, isOutput=True)

    s2dump = nc.dram_tensor("s2dump", [16, LC, L], bf16)     # [hslot, l, r]
    s3dumpT = nc.dram_tensor("s3dumpT", [L, 16, LC], fp8d)   # [r, hslot, l]
    zstage = nc.dram_tensor("zstage", [H, LC], mybir.dt.float32)

    NL2 = LC // 2   # 128 l-pairs
    NR2 = L // 2    # 512 r-pairs

    with tile.TileContext(nc) as tc:
        with (
            tc.tile_pool(name="persist", bufs=1) as pp,
            tc.tile_pool(name="work", bufs=3) as kp,
            tc.tile_pool(name="slabs", bufs=1) as sp,
            tc.tile_pool(name="finC", bufs=1) as fp,
        ):
            strm_cm = tc.tile_pool(name="strm", bufs=2)
            strm = strm_cm.__enter__()
            tp_cm = tc.tile_pool(name="tmpP", bufs=1)
            tp = tp_cm.__enter__()
            wl_cm = tc.tile_pool(name="wload", bufs=2)
            wl = wl_cm.__enter__()
            # =================== Phase P ===================
            psp_cm = tc.tile_pool(name="psP", bufs=2, space="PSUM")
            psp = psp_cm.__enter__()

            hsT_t = []
            for et in range(ET):
                t = tp.tile([128, L], bf16, tag=f"hsT{et}")
                nc.sync.dma_start(out=t, in_=hsT[et * 128:(et + 1) * 128, :])
                hsT_t.append(t)

            def load_w(handle):
                t = wl.tile([128, ET, D], bf16, tag="w6", name="w6")
                nc.sync.dma_start(
                    out=t, in_=handle[:, :].rearrange("(et p) e -> p et e", et=ET))
                return [t[:, et, :] for et in range(ET)]

            def copy_ps(dst, src, on_scalar):
                if on_scalar:
                    nc.scalar.activation(dst, src, Copy)
                else:
                    nc.vector.tensor_copy(dst, src)

            wqT_t = load_w(wqT)

            # qT [768, 256] (own chunk = hsT cols 0:256)
            qT_sb = pp.tile([128, ET, LC], bf16, tag="qT")
            for ddt in range(ET):
                ps = psp.tile([128, LC], f32, tag="ps256")
                for et in range(ET):
                    nc.tensor.matmul(
                        ps, wqT_t[et][:, ddt * 128:(ddt + 1) * 128],
                        hsT_t[et][:, 0:LC],
                        start=(et == 0), stop=(et == ET - 1),
                    )
                copy_ps(qT_sb[:, ddt, :], ps, ddt % 2 == 0)

            # rq2: [par*64+dd, hp, j, l] = rq[l, 2j+hp, dd] (both halves dup'd)
            rq2_cm = tc.tile_pool(name="rq2p", bufs=1)
            rq2pool = rq2_cm.__enter__()
            rq2 = rq2pool.tile([128, 2, ET, LC], bf16, tag="rq2")
            wkN_t = load_w(wkN)
            for j in range(ET):
                ps = psp.tile([128, LC], f32, tag="ps256")
                for et in range(ET):
                    nc.tensor.matmul(
                        ps, wkN_t[et][:, j * 128:(j + 1) * 128], qT_sb[:, et, :],
                        start=(et == 0), stop=(et == ET - 1),
                    )
                copy_ps(rq2[0:64, 0, j, :], ps[0:64, :], j % 2 == 0)
                copy_ps(rq2[64:128, 1, j, :], ps[64:128, :], j % 2 == 0)
            nc.gpsimd.dma_start(out=rq2[64:128, 0, :, :], in_=rq2[0:64, 0, :, :])
            nc.gpsimd.dma_start(out=rq2[0:64, 1, :, :], in_=rq2[64:128, 1, :, :])

            # LHS2 block-diagonal slab: col c = par*16 + h (pads zeroed only)
            LHS2 = sp.tile([128, NL2, 32], bf16, tag="LHS2")
            nc.vector.memset(LHS2[:, :, 12:16], 0.0)
            nc.vector.memset(LHS2[:, :, 28:32], 0.0)
            nc.vector.memset(LHS2[0:64, :, 16:28], 0.0)
            nc.vector.memset(LHS2[64:128, :, 0:12], 0.0)
            rq2e = rq2.rearrange("p hp j (lp two) -> p lp j hp two", two=2)
            nc.vector.tensor_copy(
                LHS2[0:64, :, 0:12].rearrange("p lp (j hp) -> p lp j hp", j=6),
                rq2e[0:64, :, :, :, 0],
            )
            nc.vector.tensor_copy(
                LHS2[64:128, :, 16:28].rearrange("p lp (j hp) -> p lp j hp", j=6),
                rq2e[64:128, :, :, :, 1],
            )
            rq2_cm.__exit__(None, None, None)

            # kT [768, 1024]
            kT_t = [pp.tile([128, L], bf16, tag=f"kT{ddt}", name=f"kT{ddt}")
                    for ddt in range(ET)]
            wkT_t = load_w(wkT)
            for ddt in range(ET):
                for nn in range(2):
                    ps = psp.tile([128, 512], f32, tag="ps512")
                    for et in range(ET):
                        nc.tensor.matmul(
                            ps, wkT_t[et][:, ddt * 128:(ddt + 1) * 128],
                            hsT_t[et][:, nn * 512:(nn + 1) * 512],
                            start=(et == 0), stop=(et == ET - 1),
                        )
                    copy_ps(kT_t[ddt][:, nn * 512:(nn + 1) * 512], ps, nn == 0)

            # v natural + ones col: V_sb [128, 8, 12, 65]
            wvT_t = load_w(wvT)
            V_sb = pp.tile([128, 8, H, d + 1], bf16, tag="V")
            nc.vector.memset(V_sb[:, :, :, d:d + 1], 1.0)
            for rt in range(8):
                psA = psp.tile([128, 512], f32, tag="ps512")
                psB = psp.tile([128, 256], f32, tag="ps256")
                for et in range(ET):
                    lw = hsT_t[et][:, rt * 128:(rt + 1) * 128]
                    nc.tensor.matmul(psA, lw, wvT_t[et][:, 0:512],
                                     start=(et == 0), stop=(et == ET - 1))
                    nc.tensor.matmul(psB, lw, wvT_t[et][:, 512:768],
                                     start=(et == 0), stop=(et == ET - 1))
                copy_ps(V_sb[:, rt, 0:8, 0:d],
                        psA.rearrange("p (h v) -> p h v", h=8), rt % 2 == 0)
                copy_ps(V_sb[:, rt, 8:12, 0:d],
                        psB.rearrange("p (h v) -> p h v", h=4), rt % 2 == 0)

            # rk2 + LHS3
            rk2_cm = tc.tile_pool(name="rk2p", bufs=1)
            rk2pool = rk2_cm.__enter__()
            rk2 = rk2pool.tile([128, 2, ET, L], bf16, tag="rk2")
            wqN_t = load_w(wqN)
            for j in range(ET):
                for nn in range(2):
                    ps2 = psp.tile([128, 512], f32, tag="ps512")
                    for et in range(ET):
                        nc.tensor.matmul(
                            ps2, wqN_t[et][:, j * 128:(j + 1) * 128],
                            kT_t[et][:, nn * 512:(nn + 1) * 512],
                            start=(et == 0), stop=(et == ET - 1),
                        )
                    copy_ps(rk2[0:64, 0, j, nn * 512:(nn + 1) * 512],
                            ps2[0:64, :], nn == 0)
                    copy_ps(rk2[64:128, 1, j, nn * 512:(nn + 1) * 512],
                            ps2[64:128, :], nn == 0)
            nc.gpsimd.dma_start(out=rk2[64:128, 0, :, :], in_=rk2[0:64, 0, :, :])
            nc.gpsimd.dma_start(out=rk2[0:64, 1, :, :], in_=rk2[64:128, 1, :, :])

            LHS3 = sp.tile([128, NR2, 32], bf16, tag="LHS3")
            nc.vector.memset(LHS3[:, :, 12:16], 0.0)
            nc.vector.memset(LHS3[:, :, 28:32], 0.0)
            nc.vector.memset(LHS3[0:64, :, 16:28], 0.0)
            nc.vector.memset(LHS3[64:128, :, 0:12], 0.0)
            rk2e = rk2.rearrange("p hp j (rp two) -> p rp j hp two", two=2)
            nc.vector.tensor_copy(
                LHS3[0:64, :, 0:12].rearrange("p rp (j hp) -> p rp j hp", j=6),
                rk2e[0:64, :, :, :, 0],
            )
            nc.vector.tensor_copy(
                LHS3[64:128, :, 16:28].rearrange("p rp (j hp) -> p rp j hp", j=6),
                rk2e[64:128, :, :, :, 1],
            )

            rk2_cm.__exit__(None, None, None)

            # C-phase constants (loaded during P; sync ring is free)
            mask_sb = fp.tile([128, 8], f32, tag="mask")
            nc.sync.dma_start(
                out=mask_sb, in_=maskc.rearrange("(t p) o -> p (t o)", p=128))
            I2 = fp.tile([128, 128], bf16, tag="I2")
            nc.sync.dma_start(out=I2, in_=ident2[:, :])
            acc = fp.tile([128, H, LC], f32, tag="acc")
            nc.vector.memset(acc[0:65, :, :], 0.0)

            # =================== Phase S2 ===================
            # dump dst: partition (s4, par, hs) -> [hs, l=8g+2*s4+par, r]
            s2dump_v = s2dump.rearrange("hs (g s p) r -> g s p hs r", s=4, p=2)
            pss_cm = tc.tile_pool(name="psS", bufs=2, space="PSUM")
            pss = pss_cm.__enter__()
            for gg in range(NL2 // 8):
                stream8 = strm.tile([128, 8, L], fp8, tag="s2stream", bufs=3)
                nc.sync.dma_start(out=stream8, in_=relS2[gg])
                for gh in range(2):
                    g = gg * 2 + gh
                    ps4 = pss.tile([128, L], f32, tag="ps1024")
                    for s4 in range(4):
                        lp = g * 4 + s4
                        for nn in range(2):
                            nc.tensor.matmul(
                                ps4[s4 * 32:(s4 + 1) * 32, nn * 512:(nn + 1) * 512],
                                LHS2[:, lp, :],
                                stream8[:, gh * 4 + s4, nn * 512:(nn + 1) * 512],
                                start=True, stop=True, tile_position=(0, 32 * s4),
                            )
                    cp = kp.tile([128, L], bf16, tag="dumpc2")
                    nc.vector.tensor_copy(cp, ps4)
                    nc.scalar.dma_start(out=s2dump_v[g], in_=cp)
            pss_cm.__exit__(None, None, None)
            psp_cm.__exit__(None, None, None)
            wl_cm.__exit__(None, None, None)
            tp_cm.__exit__(None, None, None)

            # =================== Phase S3 ===================
            s3dumpT_v = s3dumpT.rearrange("(G g s p) hs l -> G g (s p hs) l",
                                          g=4, s=4, p=2)
            scr_cm = tc.tile_pool(name="scores", bufs=4, space="PSUM")
            scr = scr_cm.__enter__()
            pvp_cm = tc.tile_pool(name="pvacc", bufs=3, space="PSUM")
            pvp = pvp_cm.__enter__()
            rl_cm = tc.tile_pool(name="rload", bufs=2)
            rl = rl_cm.__enter__()

            for G in range(NR2 // 16):
                stream16 = strm.tile([128, 16, LC], fp8, tag="s3stream", bufs=3)
                nc.sync.dma_start(out=stream16, in_=relS3[G])
                cp4 = kp.tile([128, 4, LC], fp8d, tag="dumpc3")
                for gp in range(2):
                    ps4 = scr.tile([128, 512], f32, tag="sc512", name="ps4")
                    for g4h in range(2):
                        g = G * 4 + 2 * gp + g4h
                        for s4 in range(4):
                            rp = g * 4 + s4
                            nc.tensor.matmul(
                                ps4[s4 * 32:(s4 + 1) * 32,
                                    g4h * 256:(g4h + 1) * 256],
                                LHS3[:, rp, :],
                                stream16[:, (2 * gp + g4h) * 4 + s4, :],
                                start=True, stop=True,
                                tile_position=(0, 32 * s4),
                            )
                    nc.vector.tensor_copy(
                        cp4[:, 2 * gp:2 * gp + 2, :].rearrange("p g l -> p (g l)"),
                        ps4)
                nc.scalar.dma_start(
                    out=s3dumpT_v[G].rearrange("g p l -> p g l"), in_=cp4)

            # =================== Phase C ===================
            for rt in range(8):
                s3r = rl.tile([128, H, LC], fp8d, tag="s3r")
                nc.scalar.dma_start(
                    out=s3r, in_=s3dumpT[rt * 128:(rt + 1) * 128, 0:H, :])
                s2r = rl.tile([128, H, LC], bf16, tag="s2r")
                nc.sync.dma_start_transpose(
                    out=s2r.rearrange("p hs l -> p (hs l)"),
                    in_=s2dump.rearrange("hs l r -> (hs l) r")[
                        0:H * LC, rt * 128:(rt + 1) * 128],
                )
                for h in range(H):
                    hp = h % 2
                    ps = scr.tile([128, 512], f32, tag="sc512", name="ps")[:, 0:LC]
                    nc.tensor.matmul(
                        ps,
                        kT_t[h // 2][hp * 64:(hp + 1) * 64,
                                     rt * 128:(rt + 1) * 128],
                        qT_sb[hp * 64:(hp + 1) * 64, h // 2, :],
                        start=True, stop=False, tile_position=(hp * 64, 0),
                    )
                    nc.tensor.matmul(
                        ps, I2, s2r[:, h, :],
                        start=False, stop=False, tile_position=(0, 0),
                    )
                    nc.tensor.matmul(
                        ps, I2, s3r[:, h, :],
                        start=False, stop=True, tile_position=(0, 0),
                    )
                    probs = kp.tile([128, LC], bf16, tag="probs", bufs=4)
                    nc.scalar.activation(
                        probs, ps, Exp,
                        bias=mask_sb[:, rt:rt + 1], scale=SCALE,
                    )
                    pv = pvp.tile([128, LC], f32, tag="pvps", name="pv")
                    nc.tensor.matmul(
                        pv[0:65, :], V_sb[:, rt, h, :], probs,
                        start=True, stop=True, tile_position=(0, 0),
                    )
                    nc.vector.tensor_add(
                        acc[0:65, h, :], acc[0:65, h, :], pv[0:65, :])

            rl_cm.__exit__(None, None, None)
            pvp_cm.__exit__(None, None, None)
            scr_cm.__exit__(None, None, None)
            strm_cm.__exit__(None, None, None)

            # ---------------- tail: normalize + output projection ----------
            tl_cm = tc.tile_pool(name="tailp", bufs=1)
            tl = tl_cm.__enter__()
            woT12 = tl.tile([64, H, D], bf16, tag="woT12")
            nc.sync.dma_start(
                out=woT12, in_=woT.rearrange("(h p) e -> p h e", h=H))
            zb = tl.tile([64, H, LC], f32, tag="zb")
            nc.scalar.dma_start(out=zstage[:, :], in_=acc[64:65, :, :])
            zs = zstage[:, :]
            zb_src = bass.AP(
                tensor=zs.tensor, offset=zs.offset,
                ap=[[0, 64]] + [list(x) for x in zs.ap],
            )
            nc.gpsimd.dma_start(out=zb, in_=zb_src)
            nc.vector.reciprocal(zb, zb)
            att12 = tl.tile([64, H, LC], bf16, tag="att12")
            nc.vector.tensor_mul(att12, acc[0:64, :, :], zb)

            with tc.tile_pool(name="psO", bufs=2, space="PSUM") as pso:
                for lh in range(2):
                    psA = pso.tile([128, 512], f32, tag="oA")
                    psB = pso.tile([128, 256], f32, tag="oB")
                    for h in range(H):
                        lw = att12[:, h, lh * 128:(lh + 1) * 128]
                        nc.tensor.matmul(psA, lw, woT12[:, h, 0:512],
                                         start=(h == 0), stop=(h == H - 1))
                        nc.tensor.matmul(psB, lw, woT12[:, h, 512:768],
                                         start=(h == 0), stop=(h == H - 1))
                    osb = kp.tile([128, D], f32, tag="osb", bufs=1)
                    nc.vector.tensor_copy(osb[:, 0:512], psA)
                    nc.vector.tensor_copy(osb[:, 512:768], psB)
                    nc.scalar.dma_start(
                        out=out[lh * 128:(lh + 1) * 128, :], in_=osb)

            tl_cm.__exit__(None, None, None)

    nc.compile()
    return nc


def _get_program():
    if "nc" not in _PROG_CACHE:
        _PROG_CACHE["nc"] = _build_program()
    return _PROG_CACHE["nc"]


def _host_prep(inputs):
    hs = np.asarray(inputs["hidden_states"], np.float32)
    mask = np.asarray(inputs["attention_mask"], np.float32)
    rel = np.asarray(inputs["relative_attentions"], np.float32)
    Wq = np.asarray(inputs["Wq"], np.float32)
    Wk = np.asarray(inputs["Wk"], np.float32)
    Wv = np.asarray(inputs["Wv"], np.float32)
    Wo = np.asarray(inputs["Wo"], np.float32)

    wqT = np.ascontiguousarray(Wq.T).astype(BF16)
    wkT = np.ascontiguousarray(Wk.T).astype(BF16)
    wvT = np.ascontiguousarray(Wv.T).astype(BF16)
    # halved so S2/S3 stay in fp8 range; phase C re-doubles via 2*I
    wkN = np.ascontiguousarray(Wk * 0.5).astype(BF16)
    wqN = np.ascontiguousarray(Wq * 0.5).astype(BF16)
    woT = np.ascontiguousarray(Wo.T).astype(BF16)
    ident2 = (np.eye(128, dtype=np.float32) * 2.0).astype(BF16)

    in_maps = []
    for core in range(NCORES):
        b, lci = divmod(core, 4)
        lo = lci * LC
        hs_roll = np.roll(hs[b], -lo, axis=0)
        hsT_np = np.ascontiguousarray(hs_roll.T).astype(BF16)
        rel_c = np.roll(rel[b, lo:lo + LC], -lo, axis=1)   # [256 l, 1024 r, 64 d]
        rel8 = rel_c.astype(FP8)
        # [l, r, d] -> [lp, (l2 d), r] -> [gg=16, 128, s=8, r=1024]
        relS2_np = np.ascontiguousarray(
            rel8.transpose(0, 2, 1).reshape(128, 128, L)
            .reshape(16, 8, 128, L).transpose(0, 2, 1, 3))
        # [l, r, d] -> [rp, (r2 d), l] -> [G=32, 128, s=16, l=256]
        relS3_np = np.ascontiguousarray(
            rel8.transpose(1, 2, 0).reshape(512, 128, LC)
            .reshape(32, 16, 128, LC).transpose(0, 2, 1, 3))
        maskc_np = np.ascontiguousarray(
            np.roll(mask[b, 0, 0, :], -lo).reshape(L, 1))
        in_maps.append({
            "hsT": hsT_np, "wqT": wqT, "wkT": wkT, "wvT": wvT,
            "wkN": wkN, "wqN": wqN, "woT": woT, "ident2": ident2,
            "relS2": relS2_np, "relS3": relS3_np, "maskc": maskc_np,
        })
    return in_maps


def _host_post(inputs, results):
    out = np.empty((B, L, D), np.float32)
    for core in range(NCORES):
        b, lci = divmod(core, 4)
        out[b, lci * LC:(lci + 1) * LC, :] = results[core]["out"]
    bo2 = (np.asarray(inputs["Wo"], np.float32) @ np.asarray(inputs["bv"], np.float32)
           + np.asarray(inputs["bo"], np.float32))
    out += bo2[None, None, :]
    return out


def kernel(**inputs):
    from concourse.bass_utils import run_bass_kernel_spmd

    nc = _get_program()
    in_maps = _host_prep(inputs)
    res = run_bass_kernel_spmd(nc, in_maps, list(range(NCORES)))
    return _host_post(inputs, res.results)
